# revision 34
# baseline (speedup 1.0000x reference)
"""Trainium2 Bass kernel for nn_CRF mean-field iteration (dense CRF, 5 iters).

Problem (hardcoded shapes): log_unary [1,4,32,16,16], features_pairwise
[1,2,32,16,16], compatibility = Potts (ones - eye).  N = 8192 voxels, C = 4.

Strategy (v2 — flipped DoubleRow matvec + split-AG pipeline)
------------------------------------------------------------
Same math as the baseline (see kernel_baseline.py docstring): Potts colsum
dropped, K2 applied as on-chip Kronecker factors, K1 materialized per-core
as its fp8 [8192 x 1024] column block with rowsums complete locally.

What changed vs the baseline (291 us -> 264 us):
  * Flipped matvec: u[c,n] = sum_m A[m,n] qs[m,c] keeps qs STATIONARY
    ([128,2,4] fp8, 16B-aligned pair stride) and streams A as the MOVING
    operand in DoubleRow dual-fp8 mode: 64 matmuls/iter (~300 ns each on
    HW, LDWEIGHTS serializes with the MM in dual-fp8 mode) instead of 512
    LDWEIGHTS-bound [128x128]x[128x4] matmuls (~68 ns each).  Output
    lands c-major [4,1024] and returns to n-major via 8 tiny [4,128] PE
    transposes.
  * The q exchange is TWO AllGathers per iteration (tt-halves G0/G1) in
    a (core, partition, col) layout that concatenates directly into the
    m-major q tiles (no transposes; 64/32 B DMA runs).  Emission order is
    choreographed against the in-order PE queue + 4-deep wait queue: h0's
    32 chunks close at global chunk 36, so G0's epilogue/store/AG (~6 us
    CC mesh each) overlaps h1's streaming; the separable z/y stages run
    per-half so the G0 half starts from the PREVIOUS iteration's AG0 with
    no load1 wait; h1-S0 chunks fill the load1 gap.
  * Rowsums are 64 ones-stationary DoubleRow matmuls streaming A once
    (scheduler hides them behind the materialization exp), not 512 tiny
    matmuls.
  * S1 = sqrt(reciprocal(rowsum)) (DVE reciprocal + one ACT Sqrt; Rsqrt
    ACT is blocked for accuracy), q0 ships pre-split in the (g, k, tt, c)
    m-major layout, constants ride 9 packed DMAs spread across the
    SP/ACT queues.
"""

import numpy as np
import ml_dtypes

BF16 = ml_dtypes.bfloat16
FP8 = ml_dtypes.float8_e4m3

B, C, X, Y, Z = 1, 4, 32, 16, 16
N = X * Y * Z            # 8192
P = 128                  # SBUF partitions
NCORES = 8
NB = N // NCORES         # 1024 cols per core
TM = N // P              # 64 m-tiles
TB = NB // P             # 8 block tiles
HALF = NB // 2           # 512, psum-bank-sized column half
ALPHA = 5.0              # = BETA = GAMMA in this problem
NUM_ITER = 5
W_1 = 1.0
W_2 = 1.0

_CACHE = {}
DUMMY_AG = True


def _split_hi_lo(v):
    hi = v.astype(BF16).astype(np.float32)
    lo = (v - hi).astype(BF16).astype(np.float32)
    return hi, lo


def _to_block_layout(v_nc):
    """[N, C] -> [NCORES, 128, TB*C] block-p-major device layout."""
    # n = k*NB + tt*128 + p
    return (
        v_nc.reshape(NCORES, TB, P, C).transpose(0, 2, 1, 3).reshape(NCORES, P, TB * C)
    )


def _host_constants(log_unary, features_pairwise):
    """All host-side numpy prep: layouts, constants, initial softmax."""
    lu = np.asarray(log_unary, np.float32).reshape(C, N)
    img = np.asarray(features_pairwise, np.float32).reshape(2, N)

    gx, gy, gz = np.meshgrid(
        np.arange(X), np.arange(Y), np.arange(Z), indexing="ij"
    )
    spatial = np.stack([gx, gy, gz], 0).astype(np.float32).reshape(3, N)

    f1 = np.concatenate([spatial, img], 0) / ALPHA      # [5, N]
    sq1 = (f1 * f1).sum(0)                              # [N]
    bcol = -0.5 * sq1

    f_hi, f_lo = _split_hi_lo(f1)
    b_hi, b_lo = _split_hi_lo(bcol)
    ones = np.ones((1, N), np.float32)
    # row r of lhs multiplies row r of rhs; sum over rows gives
    # f_m.f_n - .5|f_n|^2  (the -.5|f_m|^2 half is the ACT bias)
    lhs_rows = np.concatenate([f_hi, f_lo, f_hi, ones, ones], 0).astype(BF16)
    rhs_rows = np.concatenate(
        [f_hi, f_hi, f_lo, b_hi[None], b_lo[None]], 0
    ).astype(BF16)                                      # [17, N]

    bias_m = bcol.reshape(TM, P).T.copy().astype(np.float32)   # [128, 64]

    # initial q0 = softmax(lu), shipped directly in the m-major q_l layout
    e = np.exp(lu - lu.max(0, keepdims=True))
    q0 = (e / e.sum(0, keepdims=True)).T                # [N, 4]
    # [128, (g, k, tt, c)]: tt-halves (g) are the AG0/AG1 column groups
    q0m = (
        q0.reshape(NCORES, 2, 4, P, C)                  # k, g, tt, p, c
        .transpose(3, 1, 0, 2, 4)                       # p, g, k, tt, c
        .reshape(P, TM * C).astype(BF16)
    )

    # separable spatial kernel, normalization + W_2 folded into factors
    def g1d(n):
        a = np.arange(n, dtype=np.float32) / ALPHA
        return np.exp(-0.5 * (a[:, None] - a[None, :]) ** 2)

    Gx, Gy, Gz = g1d(X), g1d(Y), g1d(Z)
    gxp = Gx * (Gx.sum(1) ** -0.5)[:, None] * (Gx.sum(1) ** -0.5)[None, :]
    gyp = Gy * (Gy.sum(1) ** -0.5)[:, None] * (Gy.sum(1) ** -0.5)[None, :]
    gzp = Gz * (Gz.sum(1) ** -0.5)[:, None] * (Gz.sum(1) ** -0.5)[None, :]
    gxp *= W_2

    # Kronecker-factor constants for the on-chip pipeline
    kz = np.kron(np.eye(8, dtype=np.float32), gzp)             # [128, 128]
    ky = np.zeros((P, 4 * P), np.float32)                      # [(h*2+h')*128]
    for h in range(2):
        for hp in range(2):
            blk = np.kron(gyp[h * 8 : (h + 1) * 8, hp * 8 : (hp + 1) * 8],
                          np.eye(16, dtype=np.float32))
            ky[:, (h * 2 + hp) * P : (h * 2 + hp + 1) * P] = blk
    identity = np.eye(P, dtype=np.float32)

    lut_all = _to_block_layout(lu.T)                           # [8, 128, 32]

    in_maps = []
    for k in range(NCORES):
        blk = slice(k * NB, (k + 1) * NB)
        kx = np.kron(gxp[:, 4 * k : 4 * k + 4], np.eye(C, dtype=np.float32))
        # packed constants: bf16 pack [128, 128+512+16+128], f32 pack [128, 64+32+128]
        cpack_bf = np.concatenate(
            [kz.astype(BF16), ky.astype(BF16), kx.astype(BF16),
             identity.astype(BF16)], axis=1,
        )                                                      # [128, 784]
        cpack_f32 = np.concatenate(
            [bias_m, lut_all[k], identity], axis=1,
        )                                                      # [128, 224]
        in_maps.append(
            {
                "lhs_rows": np.ascontiguousarray(lhs_rows),
                "rhs_rows": np.ascontiguousarray(rhs_rows[:, blk]),
                "cpack_bf": np.ascontiguousarray(cpack_bf),
                "cpack_f32": np.ascontiguousarray(cpack_f32),
                "q0m": np.ascontiguousarray(q0m),
                "ones2": np.ones((P, 32), FP8),
            }
        )
    return in_maps


def _build_program():
    """Build the SPMD Bass/Tile program (same NEFF on all 8 cores)."""
    import concourse.bacc as bacc
    import concourse.mybir as mybir
    import concourse.tile as tile

    f32 = mybir.dt.float32
    bf16 = mybir.dt.bfloat16
    fp8 = mybir.dt.float8e4
    AF = mybir.ActivationFunctionType
    DR = mybir.MatmulPerfMode.DoubleRow
    RG = [list(range(NCORES))]

    nc = bacc.Bacc(
        "TRN2", target_bir_lowering=False, debug=False, num_devices=NCORES
    )

    # I/O
    lhs_rows = nc.dram_tensor("lhs_rows", [17, N], bf16, kind="ExternalInput")
    rhs_rows = nc.dram_tensor("rhs_rows", [17, NB], bf16, kind="ExternalInput")
    cpack_bf = nc.dram_tensor("cpack_bf", [P, 784], bf16, kind="ExternalInput")
    cpack_f32 = nc.dram_tensor("cpack_f32", [P, 224], f32, kind="ExternalInput")
    q0m_in = nc.dram_tensor("q0m", [P, TM * C], bf16, kind="ExternalInput")
    ones2_in = nc.dram_tensor("ones2", [P, 32], fp8, kind="ExternalInput")
    qout = nc.dram_tensor("qout", [P, TB * C], f32, kind="ExternalOutput")

    with tile.TileContext(nc) as tc:
        with (
            tc.tile_pool(name="const", bufs=1) as cp,
            tc.tile_pool(name="dram", bufs=1, space="DRAM") as dp,
        ):
            # ---- persistent SBUF tensors ----
            A_sb = cp.tile([P, TM * NB], fp8, name="A_sb")        # 64 KB/part
            lhsr_sb = cp.tile([17, N], bf16, name="lhsr_sb")
            rhsr_sb = cp.tile([17, NB], bf16, name="rhsr_sb")
            cbf_sb = cp.tile([P, 784], bf16, name="cbf_sb")
            cf32_sb = cp.tile([P, 224], f32, name="cf32_sb")
            ones2_sb = cp.tile([P, 32], fp8, name="ones2_sb")
            q_l0 = cp.tile([P, TM * C], bf16, name="q_l0")
            s1m_rep = cp.tile([P, TM * C], bf16, name="s1m_rep")
            s1n_rep = cp.tile([P, TB * C], f32, name="s1n_rep")

            # views into the packed constants
            kz_sb = cbf_sb[:, 0:128]
            ky_sb = cbf_sb[:, 128:640]
            kx_sb = cbf_sb[:, 640:656]
            idb_sb = cbf_sb[:, 656:784]
            biasm_sb = cf32_sb[:, 0:TM]
            lut_sb = cf32_sb[:, TM : TM + TB * C]
            idf_sb = cf32_sb[:, TM + TB * C : TM + TB * C + 128]

            # ---- DRAM scratch ----
            dum_in = dp.tile([512], f32, name="dum_in")
            dum_out = dp.tile([4096], f32, name="dum_out", addr_space="Shared")
            rs_blk = dp.tile([NB], f32, name="rs_blk")
            rs_full = dp.tile([N], f32, name="rs_full", addr_space="Shared")
            # per-iteration, per-n-half AG staging (split AG pipelining)
            GC = TB * C // 2  # 16 q columns per half
            qag_in = [
                [
                    dp.tile([P * GC], bf16, name=f"qag_in{i}_{g}")
                    for g in range(2)
                ]
                for i in range(4)
            ]
            qag_out = [
                [
                    dp.tile(
                        [NCORES * P * GC], bf16, name=f"qag_out{i}_{g}",
                        addr_space="Shared",
                    )
                    for g in range(2)
                ]
                for i in range(4)
            ]

            # ---- load constants (spread across idle engine queues) ----
            for ch in range(4):
                nc.sync.dma_start(
                    out=lhsr_sb[:, ch * 2048 : (ch + 1) * 2048],
                    in_=lhs_rows.ap()[:, ch * 2048 : (ch + 1) * 2048],
                )
            nc.scalar.dma_start(out=rhsr_sb[:], in_=rhs_rows.ap())
            nc.scalar.dma_start(out=cf32_sb[:], in_=cpack_f32.ap())
            nc.scalar.dma_start(out=cbf_sb[:], in_=cpack_bf.ap())
            nc.scalar.dma_start(out=ones2_sb[:], in_=ones2_in.ap())
            nc.sync.dma_start(out=q_l0[:], in_=q0m_in.ap())

            # ---- dummy collective: hides the one-time global
            #      collective-entry barrier under materialization ----
            if DUMMY_AG:
                nc.gpsimd.dma_start(
                    out=dum_in[:],
                    in_=cpack_f32.ap().rearrange("p t -> (p t)")[0:512],
                )
                nc.gpsimd.collective_compute(
                    "AllGather",
                    mybir.AluOpType.bypass,
                    replica_groups=RG,
                    ins=[dum_in[:]],
                    outs=[dum_out[:]],
                )

            A_r3 = A_sb[:].rearrange("p (t n) -> p t n", n=NB)
            # DoubleRow weights need outer (k-tile-pair) step 16B-aligned;
            # 4 ones-columns so the rowsum matmul matches the matvec's
            # (fast-path) dual-fp8 shape — rows of the output are identical.
            ones2_v = ones2_sb[:].rearrange("p (a w) -> p a w", w=16)[:, :, 0:C]

            # ================= materialization of A = K1 block =============
            # Per tile: two [17,512] bf16 matmuls + one fused exp.  The
            # column-block rowsums (complete locally) ride one tile-pair
            # behind as ones-stationary DoubleRow matmuls streaming A.
            with (
                tc.tile_pool(name="matps", bufs=2, space="PSUM") as matps,
                tc.tile_pool(name="rsps", bufs=1, space="PSUM") as rsps,
            ):
                for t in range(TM):
                    ps = matps.tile([P, NB], f32, name="mat_ps", tag="mat")
                    for h in range(2):
                        nc.tensor.matmul(
                            ps[:, h * HALF : (h + 1) * HALF],
                            lhsr_sb[:, t * P : (t + 1) * P],
                            rhsr_sb[:, h * HALF : (h + 1) * HALF],
                            start=True,
                            stop=True,
                        )
                    nc.scalar.activation(
                        A_sb[:, t * NB : (t + 1) * NB],
                        ps[:],
                        AF.Exp,
                        bias=biasm_sb[:, t : t + 1],
                    )

                # rowsum tail: back-to-back dual-fp8 matmuls streaming A
                # (only the last chunks actually wait on the trailing exps)
                rs_ps = rsps.tile([C, NB], f32, name="rs_ps")
                for j in range(TM // 2):
                    for h in range(2):
                        nc.tensor.matmul(
                            rs_ps[:, h * HALF : (h + 1) * HALF],
                            ones2_v,
                            A_r3[:, 2 * j : 2 * j + 2, h * HALF : (h + 1) * HALF],
                            start=(j == 0),
                            stop=(j == TM // 2 - 1),
                            perf_mode=DR,
                            skip_group_check=True,
                        )

                # psum -> sbuf row 0 (split DVE/ACT), then to DRAM for the AG
                rs_sb = cp.tile([1, NB], f32, name="rs_sb")
                nc.vector.tensor_copy(rs_sb[:, 0:HALF], rs_ps[0:1, 0:HALF])
                nc.scalar.activation(
                    rs_sb[:, HALF:NB], rs_ps[0:1, HALF:NB], AF.Copy
                )
            nc.sync.dma_start(
                out=rs_blk[:].rearrange("(o n) -> o n", o=1), in_=rs_sb[:]
            )
            nc.gpsimd.collective_compute(
                "AllGather",
                mybir.AluOpType.bypass,
                replica_groups=RG,
                ins=[rs_blk[:]],
                outs=[rs_full[:]],
            )

            # S1 = rsqrt(rowsum): (t,p)-major loads, PE transposes, Rsqrt.
            # The local (s1n) path starts before the AG lands.
            rsn_sb = cp.tile([TB, P], f32, name="rsn_sb")
            rsf_sb = cp.tile([TM, P], f32, name="rsf_sb")
            s1m_1 = cp.tile([P, TM], bf16, name="s1m_1")
            s1n_f = cp.tile([P, TB], f32, name="s1n_f")
            nc.scalar.dma_start(
                out=rsn_sb[:], in_=rs_blk[:].rearrange("(t p) -> t p", p=P)
            )
            nc.sync.dma_start(
                out=rsf_sb[:], in_=rs_full[:].rearrange("(t p) -> t p", p=P)
            )
            with tc.tile_pool(name="s1ps", bufs=1, space="PSUM") as s1ps:
                ntp = s1ps.tile([P, TB], f32, name="ntp", tag="ntp")
                nc.tensor.transpose(ntp[:], rsn_sb[:], idf_sb[:TB, :TB])
                rn_i = cp.tile([P, TB], f32, name="rn_i")
                nc.vector.reciprocal(rn_i[:], ntp[:])
                nc.scalar.activation(s1n_f[:], rn_i[:], AF.Sqrt)
                mtp = s1ps.tile([P, TM], f32, name="mtp", tag="mtp")
                nc.tensor.transpose(mtp[:], rsf_sb[:], idf_sb[:TM, :TM])
                rm_i = cp.tile([P, TM], f32, name="rm_i")
                nc.vector.reciprocal(rm_i[:], mtp[:])
                nc.scalar.activation(s1m_1[:], rm_i[:], AF.Sqrt)
            s1m_r3 = s1m_rep[:].rearrange("p (t c) -> p t c", c=C)
            s1n_r3 = s1n_rep[:].rearrange("p (t c) -> p t c", c=C)
            for c in range(C):
                nc.vector.tensor_copy(s1n_r3[:, :, c], s1n_f[:])
                nc.vector.tensor_copy(s1m_r3[:, :, c], s1m_1[:])

            # ======================= iterations ===========================
            # Split-AG pipeline: n-columns split into halves G0/G1 (local
            # tt 0..3 / 4..7).  The matvec streams h0's 32 m-chunks first,
            # so G0's epilogue/store/AG overlaps h1's streaming; on the
            # consumer side m-chunks sourced from AG0 (tp%4<2) run before
            # AG1-sourced ones.
            S_CHUNKS = [
                [tp for tp in range(TM // 2) if tp % 4 < 2],
                [tp for tp in range(TM // 2) if tp % 4 >= 2],
            ]
            with (
                tc.tile_pool(name="itp", bufs=2) as itp,
                tc.tile_pool(name="sep", bufs=1) as sepp,
                tc.tile_pool(name="qps", bufs=1, space="PSUM") as qpsp,
                tc.tile_pool(name="sps", bufs=2, space="PSUM") as spsp,
                tc.tile_pool(name="ups", bufs=1, space="PSUM") as upsp,
            ):
                for it in range(NUM_ITER):
                    last = it == NUM_ITER - 1

                    # -- q halves: ql_g[g] [128, (k tt c)] m-major for the
                    #    tt-half g; AG layout (k,p,ttc) concatenates in --
                    if it == 0:
                        ql_g = [q_l0[:, g * P : (g + 1) * P] for g in range(2)]
                    else:
                        ql_t = [
                            itp.tile([P, P], bf16, name=f"ql{g}", tag=f"ql{g}")
                            for g in range(2)
                        ]
                        for g in range(2):
                            nc.sync.dma_start(
                                out=ql_t[g][:].rearrange(
                                    "p (k tc) -> p k tc", k=NCORES
                                ),
                                in_=qag_out[it - 1][g][:].rearrange(
                                    "(k p tc) -> p k tc", k=NCORES, p=P
                                ),
                            )
                        ql_g = [t[:] for t in ql_t]

                    # q_s padded to 16-elem tile stride (DoubleRow weights
                    # need the outer k-tile-pair step 16B-aligned); one mul
                    # per half so S0 chunks never wait on AG1
                    q_s = itp.tile([P, TM * 16], fp8, name="q_s", tag="q_s")
                    qs_r3 = q_s[:].rearrange("p (t w) -> p t w", w=16)
                    qs_r5 = q_s[:].rearrange(
                        "p (k g tt w) -> p k g tt w", k=NCORES, g=2, w=16
                    )
                    s1m_r5 = s1m_rep[:].rearrange(
                        "p (k g tt c) -> p k g tt c", k=NCORES, g=2, c=C
                    )

                    def qmul(g):
                        nc.vector.tensor_mul(
                            qs_r5[:, :, g, :, 0:C],
                            ql_g[g].rearrange(
                                "p (k tt c) -> p k tt c", k=NCORES, c=C
                            ),
                            s1m_r5[:, :, g, :, :],
                        )

                    qmul(0)

                    # -- flipped matvec: qs stationary, A moving, DoubleRow.
                    #    Chunk order fills the load1 gap with h1-S0 work while
                    #    still closing u_h0 early (at global position 36) --
                    u_h = [
                        qpsp.tile([C, HALF], f32, name=f"u_h{h}", tag=f"uh{h}")
                        for h in range(2)
                    ]
                    chunk_list = (
                        [(0, tp) for tp in S_CHUNKS[0]]
                        + [(1, tp) for tp in S_CHUNKS[0][:8]]
                        + [(0, tp) for tp in S_CHUNKS[1]]
                        + [(1, tp) for tp in S_CHUNKS[0][8:]]
                        + [(1, tp) for tp in S_CHUNKS[1]]
                    )
                    mv_pos = [0]
                    h_cnt = [0, 0]

                    def mv(n):
                        for _ in range(n):
                            h, tp = chunk_list[mv_pos[0]]
                            nc.tensor.matmul(
                                u_h[h][:],
                                qs_r3[:, 2 * tp : 2 * tp + 2, 0:C],
                                A_r3[
                                    :,
                                    2 * tp : 2 * tp + 2,
                                    h * HALF : (h + 1) * HALF,
                                ],
                                start=(h_cnt[h] == 0),
                                stop=(h_cnt[h] == TM // 2 - 1),
                                perf_mode=DR,
                                skip_group_check=True,
                            )
                            mv_pos[0] += 1
                            h_cnt[h] += 1

                    # -- separable, split by q-half: z/y for g=0 start at
                    #    iteration 0 time (its q arrived via AG0 last iter) --
                    w2 = sepp.tile([P, 2 * X * C], bf16, name="w2")
                    w2r5 = w2[:].rearrange(
                        "p (hp k xs c) -> p hp k xs c", hp=2, k=NCORES, c=C
                    )

                    def sep_zy(g):
                        zp = spsp.tile([P, P], f32, name="zp", tag="sep")
                        nc.tensor.matmul(
                            zp[:], kz_sb, ql_g[g], start=True, stop=True
                        )
                        w1g = sepp.tile([P, P], bf16, name=f"w1g{g}",
                                        tag=f"w1g{g}")
                        nc.vector.tensor_copy(w1g[:], zp[:])
                        return w1g

                    def sep_y(g, w1g):
                        # w1g columns (k, tt, c) = (kx=(k,xs), h=tt&1, c)
                        w1r = w1g[:].rearrange(
                            "p (kx h c) -> p kx h c", h=2, c=C
                        )
                        ypg = spsp.tile([P, P], f32, name="ypg", tag="sep")
                        for hp in range(2):
                            for h in range(2):
                                nc.tensor.matmul(
                                    ypg[:, hp * 64 : (hp + 1) * 64],
                                    ky_sb[
                                        :,
                                        (h * 2 + hp) * P : (h * 2 + hp + 1) * P,
                                    ],
                                    w1r[:, :, h, :],
                                    start=(h == 0),
                                    stop=(h == 1),
                                )
                        nc.vector.tensor_copy(
                            w2r5[:, :, :, 2 * g : 2 * g + 2, :],
                            ypg[:].rearrange(
                                "p (hp k xs c) -> p hp k xs c",
                                hp=2, k=NCORES, c=C,
                            ),
                        )

                    w1g0 = sep_zy(0)
                    mv(2)
                    sep_y(0, w1g0)
                    mv(14)
                    qmul(1)
                    w1g1 = sep_zy(1)
                    mv(4)
                    sep_y(1, w1g1)
                    mv(20)
                    # 40 chunks done: u_h0 closed (16 h0S0 + 8 h1S0 + 16 h0S1)

                    # X stage + G0 tail, interleaved with h1's stream
                    q2sb = sepp.tile([P, TB * C], f32, name="q2sb")
                    q2r = q2sb[:].rearrange("p (x h c) -> p x h c", h=2, c=C)
                    lutr = lut_sb.rearrange("p (x h c) -> p x h c", h=2, c=C)
                    u_sb = sepp.tile([C, NB], f32, name="u_sb")
                    utp = upsp.tile([P, TB * C], f32, name="utp", tag="utp")
                    u_e = sepp.tile([P, TB * C], f32, name="u_e")
                    e_sb = sepp.tile([P, TB * C], f32, name="e_sb")
                    zs = sepp.tile([P, TB], f32, name="zs")
                    rz = sepp.tile([P, TB], f32, name="rz")
                    rz_rep = sepp.tile([P, TB * C], f32, name="rz_rep")
                    qn = sepp.tile(
                        [P, TB * C], f32 if last else bf16, name="qn",
                        tag="qn_f" if last else "qn_b",
                    )

                    def tail_epi(g):
                        gc = slice(g * GC, (g + 1) * GC)
                        nc.vector.tensor_mul(
                            u_e[:, gc], utp[:, gc], s1n_rep[:, gc]
                        )
                        nc.vector.tensor_add(
                            u_e[:, gc], u_e[:, gc], q2sb[:, gc]
                        )
                        nc.scalar.activation(e_sb[:, gc], u_e[:, gc], AF.Exp)
                        gt = slice(g * 4, g * 4 + 4)
                        nc.vector.reduce_sum(
                            zs[:, gt],
                            e_sb[:, gc].rearrange("p (t c) -> p t c", c=C),
                            axis=mybir.AxisListType.X,
                        )
                        nc.vector.reciprocal(rz[:, gt], zs[:, gt])
                        rzr3 = rz_rep[:, gc].rearrange("p (t c) -> p t c", c=C)
                        for c in range(C):
                            nc.vector.tensor_copy(rzr3[:, :, c], rz[:, gt])
                        nc.vector.tensor_mul(
                            qn[:, gc], e_sb[:, gc], rz_rep[:, gc]
                        )
                        if not last:
                            nc.scalar.dma_start(
                                out=qag_in[it][g][:].rearrange(
                                    "(p w) -> p w", w=GC
                                ),
                                in_=qn[:, gc],
                            )
                            nc.gpsimd.collective_compute(
                                "AllGather",
                                mybir.AluOpType.bypass,
                                replica_groups=RG,
                                ins=[qag_in[it][g][:]],
                                outs=[qag_out[it][g][:]],
                            )

                    nc.vector.tensor_copy(u_sb[:, 0:HALF], u_h[0][:])
                    for j in (0, 1):
                        nc.tensor.transpose(
                            utp[:, j * C : (j + 1) * C],
                            u_sb[:, j * P : (j + 1) * P],
                            idf_sb[:C, :C],
                        )
                    mv(2)
                    for j in (2, 3):
                        nc.tensor.transpose(
                            utp[:, j * C : (j + 1) * C],
                            u_sb[:, j * P : (j + 1) * P],
                            idf_sb[:C, :C],
                        )
                    mv(2)
                    txs, sxs = [], []
                    for hp in range(2):
                        tp1 = spsp.tile([P, P], bf16, name="tp1", tag="sep")
                        nc.tensor.transpose(
                            tp1[:], w2[:, hp * P : (hp + 1) * P], idb_sb
                        )
                        tx = sepp.tile([P, P], bf16, name="tx", tag=f"tx{hp}")
                        nc.vector.tensor_copy(tx[:], tp1[:])
                        txs.append(tx)
                        mv(2)
                    for hp in range(2):
                        xp = spsp.tile([4 * C, P], f32, name="xp", tag="sep")
                        nc.tensor.matmul(
                            xp[:], kx_sb, txs[hp][:], start=True, stop=True
                        )
                        sx = sepp.tile([4 * C, P], bf16, name="sx", tag=f"sx{hp}")
                        nc.vector.tensor_copy(sx[:], xp[:])
                        sxs.append(sx)
                        mv(2)
                    for hp in range(2):
                        tp2 = spsp.tile([P, 4 * C], bf16, name="tp2", tag="sep")
                        nc.tensor.transpose(
                            tp2[:], sxs[hp][:], idb_sb[:4 * C, :4 * C]
                        )
                        nc.vector.tensor_add(
                            q2r[:, :, hp, :],
                            tp2[:].rearrange("p (x c) -> p x c", c=C),
                            lutr[:, :, hp, :],
                        )
                        mv(2)
                    tail_epi(0)
                    mv(64 - mv_pos[0])

                    # G1 tail
                    nc.scalar.activation(u_sb[:, HALF:NB], u_h[1][:], AF.Copy)
                    for j in range(4, 8):
                        nc.tensor.transpose(
                            utp[:, j * C : (j + 1) * C],
                            u_sb[:, j * P : (j + 1) * P],
                            idf_sb[:C, :C],
                        )
                    tail_epi(1)
                    if last:
                        nc.sync.dma_start(out=qout.ap(), in_=qn[:])

    nc.compile()
    return nc


def get_program():
    if "nc" not in _CACHE:
        _CACHE["nc"] = _build_program()
    return _CACHE["nc"]


def kernel(log_unary, features_pairwise, compatibility_weights):
    import concourse.bass_utils as bass_utils

    log_unary = np.asarray(log_unary)
    features_pairwise = np.asarray(features_pairwise)
    compatibility_weights = np.asarray(compatibility_weights)
    assert log_unary.shape == (B, C, X, Y, Z)
    assert features_pairwise.shape == (B, 2, X, Y, Z)
    potts = np.ones((C, C), np.float32) - np.eye(C, dtype=np.float32)
    assert np.abs(compatibility_weights.astype(np.float32) - potts).max() < 1e-5

    in_maps = _host_constants(log_unary, features_pairwise)
    nc = get_program()
    res = bass_utils.run_bass_kernel_spmd(
        nc, in_maps, core_ids=list(range(NCORES))
    )
    # qout[k] is [128, TB*C] block-p-major; invert the layout
    q = np.stack([res.results[k]["qout"] for k in range(NCORES)], 0)
    q = q.reshape(NCORES, P, TB, C).transpose(0, 2, 1, 3).reshape(N, C)
    out = q.T.reshape(B, C, X, Y, Z).astype(np.float32)
    return out


# revision 37
# speedup vs baseline: 1.1793x; 1.1793x over previous
"""Trainium2 Bass kernel for nn_CRF mean-field iteration (dense CRF, 5 iters).

Problem (hardcoded shapes): log_unary [1,4,32,16,16], features_pairwise
[1,2,32,16,16], compatibility = Potts (ones - eye).  N = 8192 voxels, C = 4.

Strategy (v2 — flipped DoubleRow matvec + split-AG pipeline)
------------------------------------------------------------
Same math as the baseline (see kernel_baseline.py docstring): Potts colsum
dropped, K2 applied as on-chip Kronecker factors, K1 materialized per-core
as its fp8 [8192 x 1024] column block with rowsums complete locally.

What changed vs the baseline (291 us -> 264 us):
  * Flipped matvec: u[c,n] = sum_m A[m,n] qs[m,c] keeps qs STATIONARY
    ([128,2,4] fp8, 16B-aligned pair stride) and streams A as the MOVING
    operand in DoubleRow dual-fp8 mode: 64 matmuls/iter (~300 ns each on
    HW, LDWEIGHTS serializes with the MM in dual-fp8 mode) instead of 512
    LDWEIGHTS-bound [128x128]x[128x4] matmuls (~68 ns each).  Output
    lands c-major [4,1024] and returns to n-major via 8 tiny [4,128] PE
    transposes.
  * The q exchange is TWO AllGathers per iteration (tt-halves G0/G1) in
    a (core, partition, col) layout that concatenates directly into the
    m-major q tiles (no transposes; 64/32 B DMA runs).  Emission order is
    choreographed against the in-order PE queue + 4-deep wait queue: h0's
    32 chunks close at global chunk 36, so G0's epilogue/store/AG (~6 us
    CC mesh each) overlaps h1's streaming; the separable z/y stages run
    per-half so the G0 half starts from the PREVIOUS iteration's AG0 with
    no load1 wait; h1-S0 chunks fill the load1 gap.
  * Rowsums are 64 ones-stationary DoubleRow matmuls streaming A once
    (scheduler hides them behind the materialization exp), not 512 tiny
    matmuls.
  * S1 = sqrt(reciprocal(rowsum)) (DVE reciprocal + one ACT Sqrt; Rsqrt
    ACT is blocked for accuracy), q0 ships pre-split in the (g, k, tt, c)
    m-major layout, constants ride 9 packed DMAs spread across the
    SP/ACT queues.
"""

import numpy as np
import ml_dtypes

BF16 = ml_dtypes.bfloat16
FP8 = ml_dtypes.float8_e4m3

B, C, X, Y, Z = 1, 4, 32, 16, 16
N = X * Y * Z            # 8192
P = 128                  # SBUF partitions
NCORES = 8
NB = N // NCORES         # 1024 cols per core
TM = N // P              # 64 m-tiles
TB = NB // P             # 8 block tiles
HALF = NB // 2           # 512, psum-bank-sized column half
ALPHA = 5.0              # = BETA = GAMMA in this problem
NUM_ITER = 5
W_1 = 1.0
W_2 = 1.0

_CACHE = {}
DUMMY_AG = True


def _split_hi_lo(v):
    hi = v.astype(BF16).astype(np.float32)
    lo = (v - hi).astype(BF16).astype(np.float32)
    return hi, lo


def _to_block_layout(v_nc):
    """[N, C] -> [NCORES, 128, TB*C] block-p-major device layout."""
    # n = k*NB + tt*128 + p
    return (
        v_nc.reshape(NCORES, TB, P, C).transpose(0, 2, 1, 3).reshape(NCORES, P, TB * C)
    )


def _host_constants(log_unary, features_pairwise):
    """All host-side numpy prep: layouts, constants, initial softmax."""
    lu = np.asarray(log_unary, np.float32).reshape(C, N)
    img = np.asarray(features_pairwise, np.float32).reshape(2, N)

    gx, gy, gz = np.meshgrid(
        np.arange(X), np.arange(Y), np.arange(Z), indexing="ij"
    )
    spatial = np.stack([gx, gy, gz], 0).astype(np.float32).reshape(3, N)

    f1 = np.concatenate([spatial, img], 0) / ALPHA      # [5, N]
    sq1 = (f1 * f1).sum(0)                              # [N]
    bcol = -0.5 * sq1

    f_hi, f_lo = _split_hi_lo(f1)
    b_hi, b_lo = _split_hi_lo(bcol)
    ones = np.ones((1, N), np.float32)
    # row r of lhs multiplies row r of rhs; sum over rows gives
    # f_m.f_n - .5|f_n|^2  (the -.5|f_m|^2 half is the ACT bias)
    lhs_rows = np.concatenate([f_hi, f_lo, f_hi, ones, ones], 0).astype(BF16)
    rhs_rows = np.concatenate(
        [f_hi, f_hi, f_lo, b_hi[None], b_lo[None]], 0
    ).astype(BF16)                                      # [17, N]

    bias_m = bcol.reshape(TM, P).T.copy().astype(np.float32)   # [128, 64]

    # initial q0 = softmax(lu), shipped directly in the m-major q_l layout
    e = np.exp(lu - lu.max(0, keepdims=True))
    q0 = (e / e.sum(0, keepdims=True)).T                # [N, 4]
    # [128, (g, k, tt, c)]: tt-halves (g) are the AG0/AG1 column groups
    q0m = (
        q0.reshape(NCORES, 2, 4, P, C)                  # k, g, tt, p, c
        .transpose(3, 1, 0, 2, 4)                       # p, g, k, tt, c
        .reshape(P, TM * C).astype(BF16)
    )

    # separable spatial kernel, normalization + W_2 folded into factors
    def g1d(n):
        a = np.arange(n, dtype=np.float32) / ALPHA
        return np.exp(-0.5 * (a[:, None] - a[None, :]) ** 2)

    Gx, Gy, Gz = g1d(X), g1d(Y), g1d(Z)
    gxp = Gx * (Gx.sum(1) ** -0.5)[:, None] * (Gx.sum(1) ** -0.5)[None, :]
    gyp = Gy * (Gy.sum(1) ** -0.5)[:, None] * (Gy.sum(1) ** -0.5)[None, :]
    gzp = Gz * (Gz.sum(1) ** -0.5)[:, None] * (Gz.sum(1) ** -0.5)[None, :]
    gxp *= W_2

    # Kronecker-factor constants for the on-chip pipeline
    kz = np.kron(np.eye(8, dtype=np.float32), gzp)             # [128, 128]
    ky = np.zeros((P, 4 * P), np.float32)                      # [(h*2+h')*128]
    for h in range(2):
        for hp in range(2):
            blk = np.kron(gyp[h * 8 : (h + 1) * 8, hp * 8 : (hp + 1) * 8],
                          np.eye(16, dtype=np.float32))
            ky[:, (h * 2 + hp) * P : (h * 2 + hp + 1) * P] = blk
    identity = np.eye(P, dtype=np.float32)

    lut_all = _to_block_layout(lu.T)                           # [8, 128, 32]

    in_maps = []
    for k in range(NCORES):
        blk = slice(k * NB, (k + 1) * NB)
        kx = np.kron(gxp[:, 4 * k : 4 * k + 4], np.eye(C, dtype=np.float32))
        # packed constants: bf16 pack [128, 128+512+16+128], f32 pack [128, 64+32+128]
        cpack_bf = np.concatenate(
            [kz.astype(BF16), ky.astype(BF16), kx.astype(BF16),
             identity.astype(BF16)], axis=1,
        )                                                      # [128, 784]
        cpack_f32 = np.concatenate(
            [bias_m, lut_all[k], identity], axis=1,
        )                                                      # [128, 224]
        in_maps.append(
            {
                "lhs_rows": np.ascontiguousarray(lhs_rows),
                "rhs_rows": np.ascontiguousarray(rhs_rows[:, blk]),
                "cpack_bf": np.ascontiguousarray(cpack_bf),
                "cpack_f32": np.ascontiguousarray(cpack_f32),
                "q0m": np.ascontiguousarray(q0m),
                "ones2": np.ones((P, 32), FP8),
            }
        )
    return in_maps


def _build_program():
    """Build the SPMD Bass/Tile program (same NEFF on all 8 cores)."""
    import concourse.bacc as bacc
    import concourse.mybir as mybir
    import concourse.tile as tile

    f32 = mybir.dt.float32
    bf16 = mybir.dt.bfloat16
    fp8 = mybir.dt.float8e4
    AF = mybir.ActivationFunctionType
    DR = mybir.MatmulPerfMode.DoubleRow
    RG = [list(range(NCORES))]

    nc = bacc.Bacc(
        "TRN2", target_bir_lowering=False, debug=False, num_devices=NCORES
    )

    # I/O
    lhs_rows = nc.dram_tensor("lhs_rows", [17, N], bf16, kind="ExternalInput")
    rhs_rows = nc.dram_tensor("rhs_rows", [17, NB], bf16, kind="ExternalInput")
    cpack_bf = nc.dram_tensor("cpack_bf", [P, 784], bf16, kind="ExternalInput")
    cpack_f32 = nc.dram_tensor("cpack_f32", [P, 224], f32, kind="ExternalInput")
    q0m_in = nc.dram_tensor("q0m", [P, TM * C], bf16, kind="ExternalInput")
    ones2_in = nc.dram_tensor("ones2", [P, 32], fp8, kind="ExternalInput")
    qout = nc.dram_tensor("qout", [P, TB * C], f32, kind="ExternalOutput")

    with tile.TileContext(nc) as tc:
        with (
            tc.tile_pool(name="const", bufs=1) as cp,
            tc.tile_pool(name="dram", bufs=1, space="DRAM") as dp,
        ):
            # ---- persistent SBUF tensors ----
            A_sb = cp.tile([P, TM * NB], fp8, name="A_sb")        # 64 KB/part
            lhsr_sb = cp.tile([17, N], bf16, name="lhsr_sb")
            rhsr_sb = cp.tile([17, NB], bf16, name="rhsr_sb")
            cbf_sb = cp.tile([P, 784], bf16, name="cbf_sb")
            cf32_sb = cp.tile([P, 224], f32, name="cf32_sb")
            ones2_sb = cp.tile([P, 32], fp8, name="ones2_sb")
            q_l0 = cp.tile([P, TM * C], bf16, name="q_l0")
            s1m_rep = cp.tile([P, TM * C], bf16, name="s1m_rep")
            s1n_rep = cp.tile([P, TB * C], f32, name="s1n_rep")

            # views into the packed constants
            kz_sb = cbf_sb[:, 0:128]
            ky_sb = cbf_sb[:, 128:640]
            kx_sb = cbf_sb[:, 640:656]
            idb_sb = cbf_sb[:, 656:784]
            biasm_sb = cf32_sb[:, 0:TM]
            lut_sb = cf32_sb[:, TM : TM + TB * C]
            idf_sb = cf32_sb[:, TM + TB * C : TM + TB * C + 128]

            # ---- DRAM scratch ----
            dum_in = dp.tile([512], f32, name="dum_in")
            dum_out = dp.tile([4096], f32, name="dum_out", addr_space="Shared")
            rs_blk = dp.tile([NB], f32, name="rs_blk")
            rs_full = dp.tile([N], f32, name="rs_full", addr_space="Shared")
            # per-iteration, per-n-half AG staging (split AG pipelining)
            GC = TB * C // 2  # 16 q columns per half
            qag_in = [
                [
                    dp.tile([P * GC], bf16, name=f"qag_in{i}_{g}")
                    for g in range(2)
                ]
                for i in range(4)
            ]
            qag_out = [
                [
                    dp.tile(
                        [NCORES * P * GC], bf16, name=f"qag_out{i}_{g}",
                        addr_space="Shared",
                    )
                    for g in range(2)
                ]
                for i in range(4)
            ]

            # ---- load constants (spread across idle engine queues) ----
            for ch in range(4):
                nc.sync.dma_start(
                    out=lhsr_sb[:, ch * 2048 : (ch + 1) * 2048],
                    in_=lhs_rows.ap()[:, ch * 2048 : (ch + 1) * 2048],
                )
            nc.scalar.dma_start(out=rhsr_sb[:], in_=rhs_rows.ap())
            nc.scalar.dma_start(out=cf32_sb[:], in_=cpack_f32.ap())
            nc.scalar.dma_start(out=cbf_sb[:], in_=cpack_bf.ap())
            nc.scalar.dma_start(out=ones2_sb[:], in_=ones2_in.ap())
            nc.sync.dma_start(out=q_l0[:], in_=q0m_in.ap())

            # ---- dummy collective: hides the one-time global
            #      collective-entry barrier under materialization ----
            if DUMMY_AG:
                nc.gpsimd.dma_start(
                    out=dum_in[:],
                    in_=cpack_f32.ap().rearrange("p t -> (p t)")[0:512],
                )
                nc.gpsimd.collective_compute(
                    "AllGather",
                    mybir.AluOpType.bypass,
                    replica_groups=RG,
                    ins=[dum_in[:]],
                    outs=[dum_out[:]],
                )

            A_r3 = A_sb[:].rearrange("p (t n) -> p t n", n=NB)
            # DoubleRow weights need outer (k-tile-pair) step 16B-aligned;
            # 4 ones-columns so the rowsum matmul matches the matvec's
            # (fast-path) dual-fp8 shape — rows of the output are identical.
            ones2_v = ones2_sb[:].rearrange("p (a w) -> p a w", w=16)[:, :, 0:C]

            # PE p-state warm-up: junk dual-fp8 matmuls on (garbage) A
            # during the input-DMA wait, so the materialization stream
            # starts at the ramped PE clock instead of mid p-state
            with tc.tile_pool(name="wups", bufs=1, space="PSUM") as wups:
                nc.scalar.memzero(A_sb[:, 0 : 2 * NB])
                wjunk = wups.tile([C, HALF], f32, name="wjunk")
                for w in range(12):
                    nc.tensor.matmul(
                        wjunk[:],
                        ones2_v,
                        A_r3[:, 0:2, 0:HALF],
                        start=True,
                        stop=True,
                        perf_mode=DR,
                        skip_group_check=True,
                    )

            # ================= materialization of A = K1 block =============
            # Per tile: two [17,512] bf16 matmuls + one fused exp.  The
            # column-block rowsums (complete locally) ride one tile-pair
            # behind as ones-stationary DoubleRow matmuls streaming A.
            with (
                tc.tile_pool(name="matps", bufs=2, space="PSUM") as matps,
                tc.tile_pool(name="rsps", bufs=1, space="PSUM") as rsps,
            ):
                for t in range(TM):
                    ps = matps.tile([P, NB], f32, name="mat_ps", tag="mat")
                    for h in range(2):
                        nc.tensor.matmul(
                            ps[:, h * HALF : (h + 1) * HALF],
                            lhsr_sb[:, t * P : (t + 1) * P],
                            rhsr_sb[:, h * HALF : (h + 1) * HALF],
                            start=True,
                            stop=True,
                        )
                    nc.scalar.activation(
                        A_sb[:, t * NB : (t + 1) * NB],
                        ps[:],
                        AF.Exp,
                        bias=biasm_sb[:, t : t + 1],
                    )

                # rowsum tail: back-to-back dual-fp8 matmuls streaming A
                # (only the last chunks actually wait on the trailing exps)
                rs_ps = rsps.tile([C, NB], f32, name="rs_ps")
                for j in range(TM // 2):
                    for h in range(2):
                        nc.tensor.matmul(
                            rs_ps[:, h * HALF : (h + 1) * HALF],
                            ones2_v,
                            A_r3[:, 2 * j : 2 * j + 2, h * HALF : (h + 1) * HALF],
                            start=(j == 0),
                            stop=(j == TM // 2 - 1),
                            perf_mode=DR,
                            skip_group_check=True,
                        )

                # psum -> sbuf row 0 (split DVE/ACT), then to DRAM for the AG
                rs_sb = cp.tile([1, NB], f32, name="rs_sb")
                nc.vector.tensor_copy(rs_sb[:, 0:HALF], rs_ps[0:1, 0:HALF])
                nc.scalar.activation(
                    rs_sb[:, HALF:NB], rs_ps[0:1, HALF:NB], AF.Copy
                )
            nc.sync.dma_start(
                out=rs_blk[:].rearrange("(o n) -> o n", o=1), in_=rs_sb[:]
            )
            nc.gpsimd.collective_compute(
                "AllGather",
                mybir.AluOpType.bypass,
                replica_groups=RG,
                ins=[rs_blk[:]],
                outs=[rs_full[:]],
            )

            # S1 = rsqrt(rowsum): (t,p)-major loads, PE transposes, Rsqrt.
            # The local (s1n) path starts before the AG lands.
            rsn_sb = cp.tile([TB, P], f32, name="rsn_sb")
            rsf_sb = cp.tile([TM, P], f32, name="rsf_sb")
            s1m_1 = cp.tile([P, TM], bf16, name="s1m_1")
            s1n_f = cp.tile([P, TB], f32, name="s1n_f")
            nc.scalar.dma_start(
                out=rsn_sb[:], in_=rs_blk[:].rearrange("(t p) -> t p", p=P)
            )
            nc.sync.dma_start(
                out=rsf_sb[:], in_=rs_full[:].rearrange("(t p) -> t p", p=P)
            )
            with tc.tile_pool(name="s1ps", bufs=1, space="PSUM") as s1ps:
                ntp = s1ps.tile([P, TB], f32, name="ntp", tag="ntp")
                nc.tensor.transpose(ntp[:], rsn_sb[:], idf_sb[:TB, :TB])
                rn_i = cp.tile([P, TB], f32, name="rn_i")
                nc.vector.reciprocal(rn_i[:], ntp[:])
                nc.scalar.activation(s1n_f[:], rn_i[:], AF.Sqrt)
                mtp = s1ps.tile([P, TM], f32, name="mtp", tag="mtp")
                nc.tensor.transpose(mtp[:], rsf_sb[:], idf_sb[:TM, :TM])
                rm_i = cp.tile([P, TM], f32, name="rm_i")
                nc.vector.reciprocal(rm_i[:], mtp[:])
                nc.scalar.activation(s1m_1[:], rm_i[:], AF.Sqrt)
            s1m_r3 = s1m_rep[:].rearrange("p (t c) -> p t c", c=C)
            s1n_r3 = s1n_rep[:].rearrange("p (t c) -> p t c", c=C)
            for c in range(C):
                nc.vector.tensor_copy(s1n_r3[:, :, c], s1n_f[:])
                nc.vector.tensor_copy(s1m_r3[:, :, c], s1m_1[:])

            # ======================= iterations ===========================
            # Split-AG pipeline: n-columns split into halves G0/G1 (local
            # tt 0..3 / 4..7).  The matvec streams h0's 32 m-chunks first,
            # so G0's epilogue/store/AG overlaps h1's streaming; on the
            # consumer side m-chunks sourced from AG0 (tp%4<2) run before
            # AG1-sourced ones.
            S_CHUNKS = [
                [tp for tp in range(TM // 2) if tp % 4 < 2],
                [tp for tp in range(TM // 2) if tp % 4 >= 2],
            ]
            with (
                tc.tile_pool(name="itp", bufs=2) as itp,
                tc.tile_pool(name="sep", bufs=1) as sepp,
                tc.tile_pool(name="qps", bufs=1, space="PSUM") as qpsp,
                tc.tile_pool(name="sps", bufs=2, space="PSUM") as spsp,
                tc.tile_pool(name="ups", bufs=1, space="PSUM") as upsp,
            ):
                for it in range(NUM_ITER):
                    last = it == NUM_ITER - 1

                    # -- q halves: ql_g[g] [128, (k tt c)] m-major for the
                    #    tt-half g; AG layout (k,p,ttc) concatenates in --
                    if it == 0:
                        ql_g = [q_l0[:, g * P : (g + 1) * P] for g in range(2)]
                    else:
                        ql_t = [
                            itp.tile([P, P], bf16, name=f"ql{g}", tag=f"ql{g}")
                            for g in range(2)
                        ]
                        for g in range(2):
                            nc.sync.dma_start(
                                out=ql_t[g][:].rearrange(
                                    "p (k tc) -> p k tc", k=NCORES
                                ),
                                in_=qag_out[it - 1][g][:].rearrange(
                                    "(k p tc) -> p k tc", k=NCORES, p=P
                                ),
                            )
                        ql_g = [t[:] for t in ql_t]

                    # q_s padded to 16-elem tile stride (DoubleRow weights
                    # need the outer k-tile-pair step 16B-aligned); one mul
                    # per half so S0 chunks never wait on AG1
                    q_s = itp.tile([P, TM * 16], fp8, name="q_s", tag="q_s")
                    qs_r3 = q_s[:].rearrange("p (t w) -> p t w", w=16)
                    qs_r5 = q_s[:].rearrange(
                        "p (k g tt w) -> p k g tt w", k=NCORES, g=2, w=16
                    )
                    s1m_r5 = s1m_rep[:].rearrange(
                        "p (k g tt c) -> p k g tt c", k=NCORES, g=2, c=C
                    )

                    def qmul(g):
                        nc.vector.tensor_mul(
                            qs_r5[:, :, g, :, 0:C],
                            ql_g[g].rearrange(
                                "p (k tt c) -> p k tt c", k=NCORES, c=C
                            ),
                            s1m_r5[:, :, g, :, :],
                        )

                    qmul(0)

                    # -- flipped matvec: qs stationary, A moving, DoubleRow.
                    #    Chunk order fills the load1 gap with h1-S0 work while
                    #    still closing u_h0 early (at global position 36) --
                    u_h = [
                        qpsp.tile([C, HALF], f32, name=f"u_h{h}", tag=f"uh{h}")
                        for h in range(2)
                    ]
                    chunk_list = (
                        [(0, tp) for tp in S_CHUNKS[0]]
                        + [(1, tp) for tp in S_CHUNKS[0][:4]]
                        + [(0, tp) for tp in S_CHUNKS[1]]
                        + [(1, tp) for tp in S_CHUNKS[0][4:]]
                        + [(1, tp) for tp in S_CHUNKS[1]]
                    )
                    mv_pos = [0]
                    h_cnt = [0, 0]

                    def mv(n):
                        for _ in range(n):
                            h, tp = chunk_list[mv_pos[0]]
                            nc.tensor.matmul(
                                u_h[h][:],
                                qs_r3[:, 2 * tp : 2 * tp + 2, 0:C],
                                A_r3[
                                    :,
                                    2 * tp : 2 * tp + 2,
                                    h * HALF : (h + 1) * HALF,
                                ],
                                start=(h_cnt[h] == 0),
                                stop=(h_cnt[h] == TM // 2 - 1),
                                perf_mode=DR,
                                skip_group_check=True,
                            )
                            mv_pos[0] += 1
                            h_cnt[h] += 1

                    # -- separable, split by q-half: z/y for g=0 start at
                    #    iteration 0 time (its q arrived via AG0 last iter) --
                    w2 = sepp.tile([P, 2 * X * C], bf16, name="w2")
                    w2r5 = w2[:].rearrange(
                        "p (hp k xs c) -> p hp k xs c", hp=2, k=NCORES, c=C
                    )

                    def sep_zy(g):
                        zp = spsp.tile([P, P], f32, name="zp", tag="sep")
                        nc.tensor.matmul(
                            zp[:], kz_sb, ql_g[g], start=True, stop=True
                        )
                        w1g = sepp.tile([P, P], bf16, name=f"w1g{g}",
                                        tag=f"w1g{g}")
                        nc.vector.tensor_copy(w1g[:], zp[:])
                        return w1g

                    def sep_y(g, w1g):
                        # w1g columns (k, tt, c) = (kx=(k,xs), h=tt&1, c)
                        w1r = w1g[:].rearrange(
                            "p (kx h c) -> p kx h c", h=2, c=C
                        )
                        ypg = spsp.tile([P, P], f32, name="ypg", tag="sep")
                        for hp in range(2):
                            for h in range(2):
                                nc.tensor.matmul(
                                    ypg[:, hp * 64 : (hp + 1) * 64],
                                    ky_sb[
                                        :,
                                        (h * 2 + hp) * P : (h * 2 + hp + 1) * P,
                                    ],
                                    w1r[:, :, h, :],
                                    start=(h == 0),
                                    stop=(h == 1),
                                )
                        nc.vector.tensor_copy(
                            w2r5[:, :, :, 2 * g : 2 * g + 2, :],
                            ypg[:].rearrange(
                                "p (hp k xs c) -> p hp k xs c",
                                hp=2, k=NCORES, c=C,
                            ),
                        )

                    w1g0 = sep_zy(0)
                    mv(2)
                    sep_y(0, w1g0)
                    mv(14)
                    qmul(1)
                    w1g1 = sep_zy(1)
                    mv(4)
                    sep_y(1, w1g1)
                    mv(2)

                    # X stage: per-hp transpose / contract / transpose
                    q2sb = sepp.tile([P, TB * C], f32, name="q2sb")
                    q2r = q2sb[:].rearrange("p (x h c) -> p x h c", h=2, c=C)
                    lutr = lut_sb.rearrange("p (x h c) -> p x h c", h=2, c=C)
                    txs, sxs = [], []
                    for hp in range(2):
                        tp1 = spsp.tile([P, P], bf16, name="tp1", tag="sep")
                        nc.tensor.transpose(
                            tp1[:], w2[:, hp * P : (hp + 1) * P], idb_sb
                        )
                        tx = sepp.tile([P, P], bf16, name="tx", tag=f"tx{hp}")
                        nc.vector.tensor_copy(tx[:], tp1[:])
                        txs.append(tx)
                        mv(2)
                    for hp in range(2):
                        xp = spsp.tile([4 * C, P], f32, name="xp", tag="sep")
                        nc.tensor.matmul(
                            xp[:], kx_sb, txs[hp][:], start=True, stop=True
                        )
                        sx = sepp.tile([4 * C, P], bf16, name="sx", tag=f"sx{hp}")
                        nc.vector.tensor_copy(sx[:], xp[:])
                        sxs.append(sx)
                        mv(2)
                    for hp in range(2):
                        tp2 = spsp.tile([P, 4 * C], bf16, name="tp2", tag="sep")
                        nc.tensor.transpose(
                            tp2[:], sxs[hp][:], idb_sb[:4 * C, :4 * C]
                        )
                        nc.vector.tensor_add(
                            q2r[:, :, hp, :],
                            tp2[:].rearrange("p (x c) -> p x c", c=C),
                            lutr[:, :, hp, :],
                        )
                        mv(2)
                    mv(2)
                    # 36 chunks done here: u_h0 closed

                    # -- per-half tail: u -> n-major, epilogue, store, AG --
                    u_sb = sepp.tile([C, NB], f32, name="u_sb")
                    utp = upsp.tile([P, TB * C], f32, name="utp", tag="utp")
                    u_e = sepp.tile([P, TB * C], f32, name="u_e")
                    e_sb = sepp.tile([P, TB * C], f32, name="e_sb")
                    zs = sepp.tile([P, TB], f32, name="zs")
                    rz = sepp.tile([P, TB], f32, name="rz")
                    rz_rep = sepp.tile([P, TB * C], f32, name="rz_rep")
                    qn = sepp.tile(
                        [P, TB * C], f32 if last else bf16, name="qn",
                        tag="qn_f" if last else "qn_b",
                    )

                    def tail_epi(g):
                        gc = slice(g * GC, (g + 1) * GC)
                        nc.vector.tensor_mul(
                            u_e[:, gc], utp[:, gc], s1n_rep[:, gc]
                        )
                        nc.vector.tensor_add(
                            u_e[:, gc], u_e[:, gc], q2sb[:, gc]
                        )
                        nc.scalar.activation(e_sb[:, gc], u_e[:, gc], AF.Exp)
                        gt = slice(g * 4, g * 4 + 4)
                        nc.vector.reduce_sum(
                            zs[:, gt],
                            e_sb[:, gc].rearrange("p (t c) -> p t c", c=C),
                            axis=mybir.AxisListType.X,
                        )
                        nc.vector.reciprocal(rz[:, gt], zs[:, gt])
                        rzr3 = rz_rep[:, gc].rearrange("p (t c) -> p t c", c=C)
                        for c in range(C):
                            nc.vector.tensor_copy(rzr3[:, :, c], rz[:, gt])
                        nc.vector.tensor_mul(
                            qn[:, gc], e_sb[:, gc], rz_rep[:, gc]
                        )
                        if not last:
                            nc.scalar.dma_start(
                                out=qag_in[it][g][:].rearrange(
                                    "(p w) -> p w", w=GC
                                ),
                                in_=qn[:, gc],
                            )
                            nc.gpsimd.collective_compute(
                                "AllGather",
                                mybir.AluOpType.bypass,
                                replica_groups=RG,
                                ins=[qag_in[it][g][:]],
                                outs=[qag_out[it][g][:]],
                            )

                    # G0 tail interleaved into the remaining stream
                    nc.vector.tensor_copy(u_sb[:, 0:HALF], u_h[0][:])
                    for j in (0, 1):
                        nc.tensor.transpose(
                            utp[:, j * C : (j + 1) * C],
                            u_sb[:, j * P : (j + 1) * P],
                            idf_sb[:C, :C],
                        )
                    mv(4)
                    for j in (2, 3):
                        nc.tensor.transpose(
                            utp[:, j * C : (j + 1) * C],
                            u_sb[:, j * P : (j + 1) * P],
                            idf_sb[:C, :C],
                        )
                    mv(4)
                    tail_epi(0)
                    mv(64 - mv_pos[0])

                    # G1 tail
                    nc.scalar.activation(u_sb[:, HALF:NB], u_h[1][:], AF.Copy)
                    for j in range(4, 8):
                        nc.tensor.transpose(
                            utp[:, j * C : (j + 1) * C],
                            u_sb[:, j * P : (j + 1) * P],
                            idf_sb[:C, :C],
                        )
                    tail_epi(1)
                    if last:
                        nc.sync.dma_start(out=qout.ap(), in_=qn[:])

    nc.compile()
    return nc


def get_program():
    if "nc" not in _CACHE:
        _CACHE["nc"] = _build_program()
    return _CACHE["nc"]


def kernel(log_unary, features_pairwise, compatibility_weights):
    import concourse.bass_utils as bass_utils

    log_unary = np.asarray(log_unary)
    features_pairwise = np.asarray(features_pairwise)
    compatibility_weights = np.asarray(compatibility_weights)
    assert log_unary.shape == (B, C, X, Y, Z)
    assert features_pairwise.shape == (B, 2, X, Y, Z)
    potts = np.ones((C, C), np.float32) - np.eye(C, dtype=np.float32)
    assert np.abs(compatibility_weights.astype(np.float32) - potts).max() < 1e-5

    in_maps = _host_constants(log_unary, features_pairwise)
    nc = get_program()
    res = bass_utils.run_bass_kernel_spmd(
        nc, in_maps, core_ids=list(range(NCORES))
    )
    # qout[k] is [128, TB*C] block-p-major; invert the layout
    q = np.stack([res.results[k]["qout"] for k in range(NCORES)], 0)
    q = q.reshape(NCORES, P, TB, C).transpose(0, 2, 1, 3).reshape(N, C)
    out = q.T.reshape(B, C, X, Y, Z).astype(np.float32)
    return out
